# revision 1
# baseline (speedup 1.0000x reference)
"""Trainium2 Bass kernel for nn_Attention (general-score attention with
masked softmax), data-parallel over batch across 8 NeuronCores.

Math (per batch), matching the reference exactly for {0,1} float masks:
    raw[t,s]  = sum_e (hidden @ W)[t,e] * enc[s,e]       (associativity trick:
                (hidden @ W) @ enc^T  ==  hidden @ (enc @ W^T)^T, saves 25% FLOPs
                and avoids materializing proj)
    attn_energies = raw * mask            (mask in {0,1} so mask^2 == mask)
    x = attn_energies                      (softmax input; identical values)
    e = exp(x - max_s x) * mask
    attn = e / (sum_s e + 1e-6)
    context = attn @ enc_value

Layouts: host marshals hidden^T (D,T) and enc^T (E,S) per batch so every
matmul contracts over the partition dim with zero on-device transposes,
except attn^T which is produced on-device via PE transpose.
All matmuls run in float32r (e8m11, 1 cycle/row at N>=512 vs 4 for f32);
measured end-to-end rel err ~1e-3, well within tolerance.
"""
import os

import numpy as np

B, TRG, SRC, ENCD, TRGD = 16, 512, 1024, 1024, 1024
NCORES = 8
BPC = B // NCORES  # batches per core
P = 128
nD = TRGD // P   # 8 contraction tiles over d
nE = ENCD // P   # 8 over e
nS = SRC // P    # 8 over s
nT = TRG // P    # 4 t-tiles

_cache = {}

LAST_EXEC_NS = None
LAST_RESULTS = None


def _build():
    import concourse.mybir as mybir
    import concourse.tile as tile
    from concourse import bacc
    from concourse.masks import make_identity

    F32 = mybir.dt.float32
    F32R = mybir.dt.float32r
    ALU = mybir.AluOpType
    AXL = mybir.AxisListType
    ACT_EXP = mybir.ActivationFunctionType.Exp

    nc = bacc.Bacc("TRN2", target_bir_lowering=False, debug=False)

    hidT_d = nc.dram_tensor("hidT", (BPC, TRGD, TRG), F32R, kind="ExternalInput")
    w_d = nc.dram_tensor("w", (TRGD, ENCD), F32R, kind="ExternalInput")
    encT_d = nc.dram_tensor("encT", (BPC, ENCD, SRC), F32R, kind="ExternalInput")
    val_d = nc.dram_tensor("val", (BPC, SRC, TRGD), F32R, kind="ExternalInput")
    mask_d = nc.dram_tensor("mask", (BPC, 1, SRC), F32, kind="ExternalInput")
    ae_d = nc.dram_tensor("ae", (BPC, TRG, SRC), F32, kind="ExternalOutput")
    aw_d = nc.dram_tensor("aw", (BPC, TRG, SRC), F32R, kind="ExternalOutput")
    ctx_d = nc.dram_tensor("ctx", (BPC, TRG, TRGD), F32, kind="ExternalOutput")

    with tile.TileContext(nc) as tc:
        with (
            tc.tile_pool(name="const", bufs=1) as const,
            tc.tile_pool(name="big", bufs=1) as big,
            tc.tile_pool(name="sm", bufs=2) as sm,
            tc.tile_pool(name="psA", bufs=2, space="PSUM") as psA,
            tc.tile_pool(name="psB", bufs=3, space="PSUM") as psB,
        ):
            ident = const.tile([P, P], F32)
            make_identity(nc, ident[:])
            identr = const.tile([P, P], F32R)
            nc.vector.tensor_copy(identr[:], ident[:])

            # W resident for both batches: (128, nD, ENCD)
            w_sb = const.tile([P, nD, ENCD], F32R)
            for i in range(nD):
                nc.sync.dma_start(out=w_sb[:, i, :], in_=w_d[i * P:(i + 1) * P, :])

            for b in range(BPC):
                maskb = sm.tile([P, SRC], F32, tag="maskb")
                nc.sync.dma_start(out=maskb[:], in_=mask_d[b].to_broadcast((P, SRC)))

                hidT_sb = big.tile([P, nD, TRG], F32R, tag="hidT")
                for i in range(nD):
                    nc.sync.dma_start(out=hidT_sb[:, i, :],
                                      in_=hidT_d[b, i * P:(i + 1) * P, :])
                encT_sb = big.tile([P, nE, SRC], F32R, tag="encT")
                for i in range(nE):
                    nc.sync.dma_start(out=encT_sb[:, i, :],
                                      in_=encT_d[b, i * P:(i + 1) * P, :])
                val_sb = big.tile([P, nS, TRGD], F32R, tag="val")
                for i in range(nS):
                    nc.sync.dma_start(out=val_sb[:, i, :],
                                      in_=val_d[b, i * P:(i + 1) * P, :])

                # mm1: H'^T[e,t] = sum_d W[d,e] * hidden^T[d,t]
                HpT = big.tile([P, nE, TRG], F32R, tag="HpT")
                for et in range(nE):
                    pp = psA.tile([P, TRG], F32, tag="ps_a")
                    for dt in range(nD):
                        nc.tensor.matmul(pp[:], w_sb[:, dt, et * P:(et + 1) * P],
                                         hidT_sb[:, dt, :],
                                         start=(dt == 0), stop=(dt == nD - 1))
                    nc.vector.tensor_copy(HpT[:, et, :], pp[:])

                for tt in range(nT):
                    ts = slice(tt * P, (tt + 1) * P)
                    # mm2: energies[t,s] = sum_e H'^T[e,t] * enc^T[e,s]
                    en_ps = psB.tile([P, SRC], F32, tag="ps_b")
                    for et in range(nE):
                        for h in range(2):
                            hs = slice(h * 512, (h + 1) * 512)
                            nc.tensor.matmul(en_ps[:, hs], HpT[:, et, ts],
                                             encT_sb[:, et, hs],
                                             start=(et == 0), stop=(et == nE - 1))

                    # masked softmax over s (free dim)
                    x = sm.tile([P, SRC], F32, tag="x")
                    nc.vector.tensor_mul(x[:], en_ps[:], maskb[:])
                    nc.sync.dma_start(out=ae_d[b, ts, :], in_=x[:])
                    negm = sm.tile([P, 1], F32, tag="negm")
                    nc.vector.tensor_reduce(negm[:], x[:], axis=AXL.X, op=ALU.max,
                                            negate=True)
                    ex = sm.tile([P, SRC], F32, tag="ex")
                    nc.scalar.activation(ex[:], x[:], ACT_EXP, bias=negm[:], scale=1.0)
                    emask = sm.tile([P, SRC], F32, tag="emask")
                    rowsum = sm.tile([P, 1], F32, tag="rowsum")
                    nc.vector.scalar_tensor_tensor(emask[:], ex[:], 1.0, maskb[:],
                                                   op0=ALU.mult, op1=ALU.mult,
                                                   accum_out=rowsum[:])
                    z = sm.tile([P, 1], F32, tag="z")
                    nc.vector.tensor_scalar_add(z[:], rowsum[:], 1e-6)
                    rz = sm.tile([P, 1], F32, tag="rz")
                    nc.vector.reciprocal(rz[:], z[:])
                    attn = sm.tile([P, SRC], F32R, tag="attn")
                    nc.vector.tensor_scalar_mul(attn[:], emask[:], rz[:])
                    nc.sync.dma_start(out=aw_d[b, ts, :], in_=attn[:])

                    # attn^T via PE transpose, collected as (s_part, st, t)
                    attnT = big.tile([P, nS, TRG], F32R, tag="attnT")
                    for st in range(nS):
                        pt = psA.tile([P, TRG], F32R, tag="ps_a")
                        nc.tensor.transpose(pt[:, :P], attn[:, st * P:(st + 1) * P],
                                            identr[:])
                        nc.vector.tensor_copy(attnT[:, st, ts], pt[:, :P])

                    # mm3: ctx[t,d] = sum_s attn^T[s,t] * val[s,d]
                    ctx_ps = psB.tile([P, TRGD], F32, tag="ps_b")
                    for st in range(nS):
                        for h in range(2):
                            hs = slice(h * 512, (h + 1) * 512)
                            nc.tensor.matmul(ctx_ps[:, hs], attnT[:, st, ts],
                                             val_sb[:, st, hs],
                                             start=(st == 0), stop=(st == nS - 1))
                    ctx_sb = sm.tile([P, TRGD], F32, tag="ctx_sb")
                    nc.vector.tensor_copy(ctx_sb[:], ctx_ps[:])
                    nc.sync.dma_start(out=ctx_d[b, ts, :], in_=ctx_sb[:])

    nc.compile()
    return nc


def kernel(hidden, encoder_outputs, encoder_value, encoder_mask, W):
    global LAST_EXEC_NS, LAST_RESULTS
    from concourse.bass_utils import run_bass_kernel_spmd

    if "nc" not in _cache:
        _cache["nc"] = _build()
    nc = _cache["nc"]

    hidden = np.ascontiguousarray(hidden, dtype=np.float32)
    encoder_outputs = np.ascontiguousarray(encoder_outputs, dtype=np.float32)
    encoder_value = np.ascontiguousarray(encoder_value, dtype=np.float32)
    encoder_mask = np.ascontiguousarray(encoder_mask, dtype=np.float32)
    W = np.ascontiguousarray(W, dtype=np.float32)

    in_maps = []
    for c in range(NCORES):
        sl = slice(c * BPC, (c + 1) * BPC)
        in_maps.append({
            "hidT": np.ascontiguousarray(hidden[sl].transpose(0, 2, 1)),
            "w": W,
            "encT": np.ascontiguousarray(encoder_outputs[sl].transpose(0, 2, 1)),
            "val": encoder_value[sl],
            "mask": encoder_mask[sl][:, None, :],
        })

    trace = bool(int(os.environ.get("KERNEL_TRACE", "0")))
    res = run_bass_kernel_spmd(nc, in_maps, core_ids=list(range(NCORES)),
                               trace=trace)
    LAST_EXEC_NS = res.exec_time_ns
    LAST_RESULTS = res

    context = np.concatenate([res.results[c]["ctx"] for c in range(NCORES)], axis=0)
    attn_weights = np.concatenate([res.results[c]["aw"] for c in range(NCORES)],
                                  axis=0)
    attn_energies = np.concatenate([res.results[c]["ae"] for c in range(NCORES)],
                                   axis=0)
    return context, attn_weights, attn_energies


# revision 2
# speedup vs baseline: 1.5253x; 1.5253x over previous
"""Trainium2 Bass kernel for nn_Attention (general-score attention with
masked softmax), data-parallel over batch across 8 NeuronCores.

Math (per batch), matching the reference exactly for {0,1} float masks:
    raw[t,s]  = sum_e (hidden @ W)[t,e] * enc[s,e]       (associativity trick:
                (hidden @ W) @ enc^T  ==  hidden @ (enc @ W^T)^T, saves 25% FLOPs
                and avoids materializing proj)
    attn_energies = raw * mask            (mask in {0,1} so mask^2 == mask)
    x = attn_energies                      (softmax input; identical values)
    e = exp(x - max_s x) * mask
    attn = e / (sum_s e + 1e-6)
    context = attn @ enc_value

Layouts: host marshals hidden^T (D,T) and enc^T (E,S) per batch so every
matmul contracts over the partition dim with zero on-device transposes,
except attn^T which is produced on-device via PE transpose.
All matmuls run in float32r (e8m11, 1 cycle/row at N>=512 vs 4 for f32);
measured end-to-end rel err ~1e-3, well within tolerance.
"""
import os

import numpy as np

B, TRG, SRC, ENCD, TRGD = 16, 512, 1024, 1024, 1024
NCORES = 8
BPC = B // NCORES  # batches per core
P = 128
nD = TRGD // P   # 8 contraction tiles over d
nE = ENCD // P   # 8 over e
nS = SRC // P    # 8 over s
nT = TRG // P    # 4 t-tiles

_cache = {}

LAST_EXEC_NS = None
LAST_RESULTS = None


def _build():
    import concourse.mybir as mybir
    import concourse.tile as tile
    from concourse import bacc
    from concourse.masks import make_identity

    F32 = mybir.dt.float32
    F32R = mybir.dt.float32r
    ALU = mybir.AluOpType
    AXL = mybir.AxisListType
    ACT_EXP = mybir.ActivationFunctionType.Exp

    nc = bacc.Bacc("TRN2", target_bir_lowering=False, debug=False)

    hidT_d = nc.dram_tensor("hidT", (BPC, TRGD, TRG), F32R, kind="ExternalInput")
    w_d = nc.dram_tensor("w", (TRGD, ENCD), F32R, kind="ExternalInput")
    encT_d = nc.dram_tensor("encT", (BPC, ENCD, SRC), F32R, kind="ExternalInput")
    val_d = nc.dram_tensor("val", (BPC, SRC, TRGD), F32R, kind="ExternalInput")
    mask_d = nc.dram_tensor("mask", (BPC, 1, SRC), F32, kind="ExternalInput")
    ae_d = nc.dram_tensor("ae", (BPC, TRG, SRC), F32, kind="ExternalOutput")
    aw_d = nc.dram_tensor("aw", (BPC, TRG, SRC), F32R, kind="ExternalOutput")
    ctx_d = nc.dram_tensor("ctx", (BPC, TRG, TRGD), F32, kind="ExternalOutput")

    with tile.TileContext(nc) as tc:
        with (
            tc.tile_pool(name="const", bufs=1) as const,
            tc.tile_pool(name="big", bufs=1) as big,
            tc.tile_pool(name="sm", bufs=2) as sm,
            tc.tile_pool(name="psA", bufs=2, space="PSUM") as psA,
            tc.tile_pool(name="psB", bufs=3, space="PSUM") as psB,
        ):
            ident = const.tile([P, P], F32)
            make_identity(nc, ident[:])
            identr = const.tile([P, P], F32R)
            nc.vector.tensor_copy(identr[:], ident[:])

            # W resident for both batches: (128, nD, ENCD)
            w_sb = const.tile([P, nD, ENCD], F32R)
            for i in range(nD):
                nc.sync.dma_start(out=w_sb[:, i, :], in_=w_d[i * P:(i + 1) * P, :])

            for b in range(BPC):
                maskb = sm.tile([P, SRC], F32, tag="maskb")
                nc.sync.dma_start(out=maskb[:], in_=mask_d[b].to_broadcast((P, SRC)))

                hidT_sb = big.tile([P, nD, TRG], F32R, tag="hidT")
                for i in range(nD):
                    nc.sync.dma_start(out=hidT_sb[:, i, :],
                                      in_=hidT_d[b, i * P:(i + 1) * P, :])
                encT_sb = big.tile([P, nE, SRC], F32R, tag="encT")
                for i in range(nE):
                    nc.sync.dma_start(out=encT_sb[:, i, :],
                                      in_=encT_d[b, i * P:(i + 1) * P, :])
                val_sb = big.tile([P, nS, TRGD], F32R, tag="val")
                for i in range(nS):
                    nc.sync.dma_start(out=val_sb[:, i, :],
                                      in_=val_d[b, i * P:(i + 1) * P, :])

                # mm1: H'^T[e,t] = sum_d W[d,e] * hidden^T[d,t]
                HpT = big.tile([P, nE, TRG], F32R, tag="HpT")
                for et in range(nE):
                    pp = psA.tile([P, TRG], F32, tag="ps_a")
                    for dt in range(nD):
                        nc.tensor.matmul(pp[:], w_sb[:, dt, et * P:(et + 1) * P],
                                         hidT_sb[:, dt, :],
                                         start=(dt == 0), stop=(dt == nD - 1))
                    nc.vector.tensor_copy(HpT[:, et, :], pp[:])

                def emit_mm2(tt):
                    ts = slice(tt * P, (tt + 1) * P)
                    en_ps = psB.tile([P, SRC], F32, tag="ps_b")
                    for et in range(nE):
                        for h in range(2):
                            hs = slice(h * 512, (h + 1) * 512)
                            nc.tensor.matmul(en_ps[:, hs], HpT[:, et, ts],
                                             encT_sb[:, et, hs],
                                             start=(et == 0), stop=(et == nE - 1))
                    return en_ps

                # software pipeline over t-tiles: mm2(tt+1) is emitted between
                # softmax(tt) and the transpose/mm3 of tt so the PE never
                # stalls on the softmax chain (keeps HAM warm, too).
                en_ps = emit_mm2(0)
                for tt in range(nT):
                    ts = slice(tt * P, (tt + 1) * P)

                    # masked softmax over s (free dim)
                    x = sm.tile([P, SRC], F32, tag="x")
                    nc.vector.tensor_mul(x[:], en_ps[:], maskb[:])
                    nc.sync.dma_start(out=ae_d[b, ts, :], in_=x[:])
                    negm = sm.tile([P, 1], F32, tag="negm")
                    nc.vector.tensor_reduce(negm[:], x[:], axis=AXL.X, op=ALU.max,
                                            negate=True)
                    ex = sm.tile([P, SRC], F32, tag="ex")
                    nc.scalar.activation(ex[:], x[:], ACT_EXP, bias=negm[:], scale=1.0)
                    emask = sm.tile([P, SRC], F32, tag="emask")
                    rowsum = sm.tile([P, 1], F32, tag="rowsum")
                    nc.vector.scalar_tensor_tensor(emask[:], ex[:], 1.0, maskb[:],
                                                   op0=ALU.mult, op1=ALU.mult,
                                                   accum_out=rowsum[:])
                    z = sm.tile([P, 1], F32, tag="z")
                    nc.vector.tensor_scalar_add(z[:], rowsum[:], 1e-6)
                    rz = sm.tile([P, 1], F32, tag="rz")
                    nc.vector.reciprocal(rz[:], z[:])
                    attn = sm.tile([P, SRC], F32R, tag="attn")
                    nc.vector.tensor_scalar_mul(attn[:], emask[:], rz[:])
                    nc.sync.dma_start(out=aw_d[b, ts, :], in_=attn[:])

                    if tt + 1 < nT:
                        en_ps = emit_mm2(tt + 1)

                    # attn^T via PE transpose, per-tt tiles (s_part, st, t128)
                    attnT = sm.tile([P, nS, P], F32R, tag="attnT")
                    for st in range(nS):
                        pt = psA.tile([P, TRG], F32R, tag="ps_a")
                        nc.tensor.transpose(pt[:, :P], attn[:, st * P:(st + 1) * P],
                                            identr[:])
                        if st % 2 == 0:
                            nc.vector.tensor_copy(attnT[:, st, :], pt[:, :P])
                        else:
                            nc.scalar.copy(attnT[:, st, :], pt[:, :P])

                    # mm3: ctx[t,d] = sum_s attn^T[s,t] * val[s,d]
                    ctx_ps = psB.tile([P, TRGD], F32, tag="ps_b")
                    for st in range(nS):
                        for h in range(2):
                            hs = slice(h * 512, (h + 1) * 512)
                            nc.tensor.matmul(ctx_ps[:, hs], attnT[:, st, :],
                                             val_sb[:, st, hs],
                                             start=(st == 0), stop=(st == nS - 1))
                    ctx_sb = sm.tile([P, TRGD], F32, tag="ctx_sb")
                    nc.scalar.copy(ctx_sb[:], ctx_ps[:])
                    nc.sync.dma_start(out=ctx_d[b, ts, :], in_=ctx_sb[:])

    nc.compile()
    return nc


def kernel(hidden, encoder_outputs, encoder_value, encoder_mask, W):
    global LAST_EXEC_NS, LAST_RESULTS
    from concourse.bass_utils import run_bass_kernel_spmd

    if "nc" not in _cache:
        _cache["nc"] = _build()
    nc = _cache["nc"]

    hidden = np.ascontiguousarray(hidden, dtype=np.float32)
    encoder_outputs = np.ascontiguousarray(encoder_outputs, dtype=np.float32)
    encoder_value = np.ascontiguousarray(encoder_value, dtype=np.float32)
    encoder_mask = np.ascontiguousarray(encoder_mask, dtype=np.float32)
    W = np.ascontiguousarray(W, dtype=np.float32)

    in_maps = []
    for c in range(NCORES):
        sl = slice(c * BPC, (c + 1) * BPC)
        in_maps.append({
            "hidT": np.ascontiguousarray(hidden[sl].transpose(0, 2, 1)),
            "w": W,
            "encT": np.ascontiguousarray(encoder_outputs[sl].transpose(0, 2, 1)),
            "val": encoder_value[sl],
            "mask": encoder_mask[sl][:, None, :],
        })

    trace = bool(int(os.environ.get("KERNEL_TRACE", "0")))
    res = run_bass_kernel_spmd(nc, in_maps, core_ids=list(range(NCORES)),
                               trace=trace)
    LAST_EXEC_NS = res.exec_time_ns
    LAST_RESULTS = res

    context = np.concatenate([res.results[c]["ctx"] for c in range(NCORES)], axis=0)
    attn_weights = np.concatenate([res.results[c]["aw"] for c in range(NCORES)],
                                  axis=0)
    attn_energies = np.concatenate([res.results[c]["ae"] for c in range(NCORES)],
                                   axis=0)
    return context, attn_weights, attn_energies


# revision 5
# speedup vs baseline: 1.5686x; 1.0284x over previous
"""Trainium2 Bass kernel for nn_Attention (general-score attention with
masked softmax), data-parallel over batch across 8 NeuronCores.

Math (per batch), matching the reference exactly for {0,1} float masks:
    raw[t,s]  = sum_e (hidden @ W)[t,e] * enc[s,e]       (associativity trick:
                (hidden @ W) @ enc^T  ==  hidden @ (enc @ W^T)^T, saves 25% FLOPs
                and avoids materializing proj)
    attn_energies = raw * mask            (mask in {0,1} so mask^2 == mask)
    e = exp(x - max_s x) * mask
    attn = e / (sum_s e + 1e-6)
    context = attn @ enc_value

Layouts: host marshals hidden^T (D,T) and enc^T (E,S) per batch so every
matmul contracts over the partition dim with zero on-device transposes,
except attn^T which is produced on-device via PE transpose.
All matmuls run in float32r (e8m11, 1 cycle/row at N>=512 vs 4 for f32);
measured end-to-end rel err ~1.2e-3.

Schedule: mm1 (per-dt-sliced loads for fast ramp) -> mm2 for ALL t-tiles
(freeing enc^T for the next batch mid-batch) with the softmax chains
pipelined on DVE/ACT behind the mm2 stream -> per t-tile PE transposes +
mm3. The PE stream never waits on a softmax chain in steady state.
"""
import os

import numpy as np

B, TRG, SRC, ENCD, TRGD = 16, 512, 1024, 1024, 1024
NCORES = 8
BPC = B // NCORES  # batches per core
P = 128
nD = TRGD // P   # 8 contraction tiles over d
nE = ENCD // P   # 8 over e
nS = SRC // P    # 8 over s
nT = TRG // P    # 4 t-tiles

_cache = {}

LAST_EXEC_NS = None
LAST_RESULTS = None


def _build():
    import concourse.mybir as mybir
    import concourse.tile as tile
    from concourse import bacc
    from concourse.masks import make_identity

    F32 = mybir.dt.float32
    F32R = mybir.dt.float32r
    ALU = mybir.AluOpType
    AXL = mybir.AxisListType
    ACT_EXP = mybir.ActivationFunctionType.Exp

    nc = bacc.Bacc("TRN2", target_bir_lowering=False, debug=False)

    hidT_d = nc.dram_tensor("hidT", (BPC, TRGD, TRG), F32R, kind="ExternalInput")
    w_d = nc.dram_tensor("w", (TRGD, ENCD), F32R, kind="ExternalInput")
    encT_d = nc.dram_tensor("encT", (BPC, ENCD, SRC), F32R, kind="ExternalInput")
    val_d = nc.dram_tensor("val", (BPC, SRC, TRGD), F32R, kind="ExternalInput")
    mask_d = nc.dram_tensor("mask", (BPC, 1, SRC), F32, kind="ExternalInput")
    ae_d = nc.dram_tensor("ae", (BPC, TRG, SRC), F32, kind="ExternalOutput")
    aw_d = nc.dram_tensor("aw", (BPC, TRG, SRC), F32R, kind="ExternalOutput")
    ctx_d = nc.dram_tensor("ctx", (BPC, TRG, TRGD), F32, kind="ExternalOutput")

    with tile.TileContext(nc) as tc:
        with (
            tc.tile_pool(name="const", bufs=1) as const,
            tc.tile_pool(name="wp", bufs=1) as wp,
            tc.tile_pool(name="big", bufs=1) as big,
            tc.tile_pool(name="sm", bufs=2) as sm,
            tc.tile_pool(name="xs", bufs=4) as xs,
            tc.tile_pool(name="psA", bufs=2, space="PSUM") as psA,
            tc.tile_pool(name="psB", bufs=3, space="PSUM") as psB,
        ):
            ident = const.tile([P, P], F32)
            make_identity(nc, ident[:])
            identr = const.tile([P, P], F32R)
            nc.vector.tensor_copy(identr[:], ident[:])

            # W resident for both batches, per-dt tiles for fast first-mm ramp
            w_sb = [wp.tile([P, ENCD], F32R, tag=f"w{i}", name=f"w_sb{i}")
                    for i in range(nD)]
            for i in range(nD):
                nc.sync.dma_start(out=w_sb[i][:], in_=w_d[i * P:(i + 1) * P, :])

            for b in range(BPC):
                maskb = sm.tile([P, SRC], F32, tag="maskb")
                nc.sync.dma_start(out=maskb[:], in_=mask_d[b].to_broadcast((P, SRC)))

                hidT_sb = [big.tile([P, TRG], F32R, tag=f"hidT{i}",
                                     name=f"hidT_sb{i}") for i in range(nD)]
                for i in range(nD):
                    nc.sync.dma_start(out=hidT_sb[i][:],
                                      in_=hidT_d[b, i * P:(i + 1) * P, :])
                encT_sb = big.tile([P, nE, SRC], F32R, tag="encT")
                for i in range(nE):
                    nc.sync.dma_start(out=encT_sb[:, i, :],
                                      in_=encT_d[b, i * P:(i + 1) * P, :])
                val_sb = big.tile([P, nS, TRGD], F32R, tag="val")
                for i in range(nS):
                    nc.sync.dma_start(out=val_sb[:, i, :],
                                      in_=val_d[b, i * P:(i + 1) * P, :])

                # mm1: H'^T[e,t] = sum_d W[d,e] * hidden^T[d,t]
                HpT = big.tile([P, nE, TRG], F32R, tag="HpT")
                for et in range(nE):
                    pp = psA.tile([P, TRG], F32, tag="ps_a")
                    for dt in range(nD):
                        nc.tensor.matmul(pp[:], w_sb[dt][:, et * P:(et + 1) * P],
                                         hidT_sb[dt][:],
                                         start=(dt == 0), stop=(dt == nD - 1))
                    nc.vector.tensor_copy(HpT[:, et, :], pp[:])

                # mm2 for all t-tiles; softmax chains pipeline behind the
                # PE stream on DVE/ACT.
                attns = []
                for tt in range(nT):
                    ts = slice(tt * P, (tt + 1) * P)
                    en_ps = psB.tile([P, SRC], F32, tag="ps_b")
                    for et in range(nE):
                        for h in range(2):
                            hs = slice(h * 512, (h + 1) * 512)
                            nc.tensor.matmul(en_ps[:, hs], HpT[:, et, ts],
                                             encT_sb[:, et, hs],
                                             start=(et == 0), stop=(et == nE - 1))

                    x = xs.tile([P, SRC], F32, tag="x")
                    nc.vector.tensor_mul(x[:], en_ps[:], maskb[:])
                    nc.sync.dma_start(out=ae_d[b, ts, :], in_=x[:])
                    negm = sm.tile([P, 1], F32, tag="negm")
                    nc.vector.tensor_reduce(negm[:], x[:], axis=AXL.X, op=ALU.max,
                                            negate=True)
                    ex = sm.tile([P, SRC], F32, tag="ex")
                    nc.scalar.activation(ex[:], x[:], ACT_EXP, bias=negm[:],
                                         scale=1.0)
                    rowsum = sm.tile([P, 1], F32, tag="rowsum")
                    nc.vector.scalar_tensor_tensor(ex[:], ex[:], 1.0, maskb[:],
                                                   op0=ALU.mult, op1=ALU.mult,
                                                   accum_out=rowsum[:])
                    emask = ex
                    z = sm.tile([P, 1], F32, tag="z")
                    nc.vector.tensor_scalar_add(z[:], rowsum[:], 1e-6)
                    rz = sm.tile([P, 1], F32, tag="rz")
                    nc.vector.reciprocal(rz[:], z[:])
                    attn = xs.tile([P, SRC], F32R, tag="attn")
                    nc.vector.tensor_scalar_mul(attn[:], emask[:], rz[:])
                    nc.sync.dma_start(out=aw_d[b, ts, :], in_=attn[:])
                    attns.append(attn)

                # per t-tile: attn^T via PE transpose, then mm3
                for tt in range(nT):
                    ts = slice(tt * P, (tt + 1) * P)
                    attn = attns[tt]
                    attnT = sm.tile([P, nS, P], F32R, tag="attnT")
                    for st in range(nS):
                        pt = psA.tile([P, TRG], F32R, tag="ps_a")
                        nc.tensor.transpose(pt[:, :P], attn[:, st * P:(st + 1) * P],
                                            identr[:])
                        if st % 2 == 0:
                            nc.vector.tensor_copy(attnT[:, st, :], pt[:, :P])
                        else:
                            nc.scalar.copy(attnT[:, st, :], pt[:, :P])

                    ctx_ps = psB.tile([P, TRGD], F32, tag="ps_b")
                    for st in range(nS):
                        for h in range(2):
                            hs = slice(h * 512, (h + 1) * 512)
                            nc.tensor.matmul(ctx_ps[:, hs], attnT[:, st, :],
                                             val_sb[:, st, hs],
                                             start=(st == 0), stop=(st == nS - 1))
                    ctx_sb = sm.tile([P, TRGD], F32, tag="ctx_sb")
                    nc.scalar.copy(ctx_sb[:], ctx_ps[:])
                    nc.sync.dma_start(out=ctx_d[b, ts, :], in_=ctx_sb[:])

    nc.compile()
    return nc


def kernel(hidden, encoder_outputs, encoder_value, encoder_mask, W):
    global LAST_EXEC_NS, LAST_RESULTS
    from concourse.bass_utils import run_bass_kernel_spmd

    if "nc" not in _cache:
        _cache["nc"] = _build()
    nc = _cache["nc"]

    hidden = np.ascontiguousarray(hidden, dtype=np.float32)
    encoder_outputs = np.ascontiguousarray(encoder_outputs, dtype=np.float32)
    encoder_value = np.ascontiguousarray(encoder_value, dtype=np.float32)
    encoder_mask = np.ascontiguousarray(encoder_mask, dtype=np.float32)
    W = np.ascontiguousarray(W, dtype=np.float32)

    in_maps = []
    for c in range(NCORES):
        sl = slice(c * BPC, (c + 1) * BPC)
        in_maps.append({
            "hidT": np.ascontiguousarray(hidden[sl].transpose(0, 2, 1)),
            "w": W,
            "encT": np.ascontiguousarray(encoder_outputs[sl].transpose(0, 2, 1)),
            "val": encoder_value[sl],
            "mask": encoder_mask[sl][:, None, :],
        })

    trace = bool(int(os.environ.get("KERNEL_TRACE", "0")))
    res = run_bass_kernel_spmd(nc, in_maps, core_ids=list(range(NCORES)),
                               trace=trace)
    LAST_EXEC_NS = res.exec_time_ns
    LAST_RESULTS = res

    context = np.concatenate([res.results[c]["ctx"] for c in range(NCORES)], axis=0)
    attn_weights = np.concatenate([res.results[c]["aw"] for c in range(NCORES)],
                                  axis=0)
    attn_energies = np.concatenate([res.results[c]["ae"] for c in range(NCORES)],
                                   axis=0)
    return context, attn_weights, attn_energies


# revision 6
# speedup vs baseline: 1.6164x; 1.0305x over previous
"""Trainium2 Bass kernel for nn_Attention (general-score attention with
masked softmax), data-parallel over batch across 8 NeuronCores.

Math (per batch), matching the reference exactly for {0,1} float masks:
    raw[t,s]  = sum_e (hidden @ W)[t,e] * enc[s,e]       (associativity trick:
                (hidden @ W) @ enc^T  ==  hidden @ (enc @ W^T)^T, saves 25% FLOPs
                and avoids materializing proj)
    attn_energies = raw * mask            (mask in {0,1} so mask^2 == mask)
    e = exp(x - max_s x) * mask
    attn = e / (sum_s e + 1e-6)
    context = attn @ enc_value

Layouts: host marshals hidden^T (D,T) and enc^T (E,S) per batch so every
matmul contracts over the partition dim with zero on-device transposes,
except attn^T which is produced on-device via PE transpose.
All matmuls run in float32r (e8m11, 1 cycle/row at N>=512 vs 4 for f32);
measured end-to-end rel err ~1.2e-3.

Schedule: mm1 (per-dt-sliced loads for fast ramp) -> mm2 for ALL t-tiles
(freeing enc^T for the next batch mid-batch) with the softmax chains
pipelined on DVE/ACT behind the mm2 stream -> per t-tile PE transposes +
mm3. The PE stream never waits on a softmax chain in steady state.
"""
import os

import numpy as np

B, TRG, SRC, ENCD, TRGD = 16, 512, 1024, 1024, 1024
NCORES = 8
BPC = B // NCORES  # batches per core
P = 128
nD = TRGD // P   # 8 contraction tiles over d
nE = ENCD // P   # 8 over e
nS = SRC // P    # 8 over s
nT = TRG // P    # 4 t-tiles

_cache = {}

LAST_EXEC_NS = None
LAST_RESULTS = None


def _build():
    import concourse.mybir as mybir
    import concourse.tile as tile
    from concourse import bacc
    from concourse.masks import make_identity

    F32 = mybir.dt.float32
    F32R = mybir.dt.float32r
    ALU = mybir.AluOpType
    AXL = mybir.AxisListType
    ACT_EXP = mybir.ActivationFunctionType.Exp

    nc = bacc.Bacc("TRN2", target_bir_lowering=False, debug=False)

    hidT_d = nc.dram_tensor("hidT", (BPC, TRGD, TRG), F32R, kind="ExternalInput")
    w_d = nc.dram_tensor("w", (TRGD, ENCD), F32R, kind="ExternalInput")
    encT_d = nc.dram_tensor("encT", (BPC, ENCD, SRC), F32R, kind="ExternalInput")
    val_d = nc.dram_tensor("val", (BPC, SRC, TRGD), F32R, kind="ExternalInput")
    mask_d = nc.dram_tensor("mask", (BPC, 1, SRC), F32, kind="ExternalInput")
    ae_d = nc.dram_tensor("ae", (BPC, TRG, SRC), F32, kind="ExternalOutput")
    aw_d = nc.dram_tensor("aw", (BPC, TRG, SRC), F32R, kind="ExternalOutput")
    ctx_d = nc.dram_tensor("ctx", (BPC, TRG, TRGD), F32, kind="ExternalOutput")

    with tile.TileContext(nc) as tc:
        with (
            tc.tile_pool(name="const", bufs=1) as const,
            tc.tile_pool(name="wp", bufs=1) as wp,
            tc.tile_pool(name="big", bufs=1) as big,
            tc.tile_pool(name="sm", bufs=2) as sm,
            tc.tile_pool(name="xs", bufs=4) as xs,
            tc.tile_pool(name="psA", bufs=2, space="PSUM") as psA,
            tc.tile_pool(name="psB", bufs=3, space="PSUM") as psB,
        ):
            ident = const.tile([P, P], F32)
            make_identity(nc, ident[:])
            identr = const.tile([P, P], F32R)
            nc.vector.tensor_copy(identr[:], ident[:])

            # W resident for both batches, per-dt tiles for fast first-mm ramp
            w_sb = [wp.tile([P, ENCD], F32R, tag=f"w{i}", name=f"w_sb{i}")
                    for i in range(nD)]

            for b in range(BPC):
                hidT_sb = [big.tile([P, TRG], F32R, tag=f"hidT{i}",
                                     name=f"hidT_sb{i}") for i in range(nD)]
                # DMA issue order == consumption order: (w[i], hidT[i]) pairs
                # feed the dt-outer mm1 below as they land.
                for i in range(nD):
                    if b == 0:
                        nc.sync.dma_start(out=w_sb[i][:],
                                          in_=w_d[i * P:(i + 1) * P, :])
                    nc.sync.dma_start(out=hidT_sb[i][:],
                                      in_=hidT_d[b, i * P:(i + 1) * P, :])
                maskb = sm.tile([P, SRC], F32, tag="maskb")
                nc.sync.dma_start(out=maskb[:], in_=mask_d[b].to_broadcast((P, SRC)))
                encT_sb = big.tile([P, nE, SRC], F32R, tag="encT")
                for i in range(nE):
                    nc.sync.dma_start(out=encT_sb[:, i, :],
                                      in_=encT_d[b, i * P:(i + 1) * P, :])
                val_sb = big.tile([P, nS, TRGD], F32R, tag="val")
                for i in range(nS):
                    nc.sync.dma_start(out=val_sb[:, i, :],
                                      in_=val_d[b, i * P:(i + 1) * P, :])

                # mm1: H'^T[e,t] = sum_d W[d,e] * hidden^T[d,t], dt-OUTER over
                # 8 concurrent psum banks so each (w[dt], hidT[dt]) slice pair
                # is consumed as soon as its DMA lands.
                HpT = big.tile([P, nE, TRG], F32R, tag="HpT")
                mm1_ps = [psB.tile([P, SRC], F32, tag="ps_b",
                                   name=f"mm1ps{j}") for j in range(3)]
                mm1_ps2 = [psA.tile([P, TRG], F32, tag="ps_a",
                                    name=f"mm1ps2{j}") for j in range(2)]

                def et_psum(et):
                    if et < 6:
                        return mm1_ps[et // 2][:, (et % 2) * 512:(et % 2 + 1) * 512]
                    return mm1_ps2[et - 6][:]

                for dt in range(nD):
                    for et in range(nE):
                        nc.tensor.matmul(et_psum(et),
                                         w_sb[dt][:, et * P:(et + 1) * P],
                                         hidT_sb[dt][:],
                                         start=(dt == 0), stop=(dt == nD - 1))
                for et in range(nE):
                    nc.vector.tensor_copy(HpT[:, et, :], et_psum(et))

                # mm2 for all t-tiles; softmax chains pipeline behind the
                # PE stream on DVE/ACT.
                attns = []
                for tt in range(nT):
                    ts = slice(tt * P, (tt + 1) * P)
                    en_ps = psB.tile([P, SRC], F32, tag="ps_b")
                    for et in range(nE):
                        for h in range(2):
                            hs = slice(h * 512, (h + 1) * 512)
                            nc.tensor.matmul(en_ps[:, hs], HpT[:, et, ts],
                                             encT_sb[:, et, hs],
                                             start=(et == 0), stop=(et == nE - 1))

                    x = xs.tile([P, SRC], F32, tag="x")
                    nc.vector.tensor_mul(x[:], en_ps[:], maskb[:])
                    nc.sync.dma_start(out=ae_d[b, ts, :], in_=x[:])
                    negm = sm.tile([P, 1], F32, tag="negm")
                    nc.vector.tensor_reduce(negm[:], x[:], axis=AXL.X, op=ALU.max,
                                            negate=True)
                    ex = sm.tile([P, SRC], F32, tag="ex")
                    nc.scalar.activation(ex[:], x[:], ACT_EXP, bias=negm[:],
                                         scale=1.0)
                    rowsum = sm.tile([P, 1], F32, tag="rowsum")
                    nc.vector.scalar_tensor_tensor(ex[:], ex[:], 1.0, maskb[:],
                                                   op0=ALU.mult, op1=ALU.mult,
                                                   accum_out=rowsum[:])
                    emask = ex
                    z = sm.tile([P, 1], F32, tag="z")
                    nc.vector.tensor_scalar_add(z[:], rowsum[:], 1e-6)
                    rz = sm.tile([P, 1], F32, tag="rz")
                    nc.vector.reciprocal(rz[:], z[:])
                    attn = xs.tile([P, SRC], F32R, tag="attn")
                    nc.vector.tensor_scalar_mul(attn[:], emask[:], rz[:])
                    nc.sync.dma_start(out=aw_d[b, ts, :], in_=attn[:])
                    attns.append(attn)

                # per t-tile: attn^T via PE transpose, then mm3
                for tt in range(nT):
                    ts = slice(tt * P, (tt + 1) * P)
                    attn = attns[tt]
                    attnT = sm.tile([P, nS, P], F32R, tag="attnT")
                    for st in range(nS):
                        pt = psA.tile([P, TRG], F32R, tag="ps_a")
                        nc.tensor.transpose(pt[:, :P], attn[:, st * P:(st + 1) * P],
                                            identr[:])
                        if st % 2 == 0:
                            nc.vector.tensor_copy(attnT[:, st, :], pt[:, :P])
                        else:
                            nc.scalar.copy(attnT[:, st, :], pt[:, :P])

                    ctx_ps = psB.tile([P, TRGD], F32, tag="ps_b")
                    for st in range(nS):
                        for h in range(2):
                            hs = slice(h * 512, (h + 1) * 512)
                            nc.tensor.matmul(ctx_ps[:, hs], attnT[:, st, :],
                                             val_sb[:, st, hs],
                                             start=(st == 0), stop=(st == nS - 1))
                    ctx_sb = sm.tile([P, TRGD], F32, tag="ctx_sb")
                    nc.scalar.copy(ctx_sb[:], ctx_ps[:])
                    nc.sync.dma_start(out=ctx_d[b, ts, :], in_=ctx_sb[:])

    nc.compile()
    return nc


def kernel(hidden, encoder_outputs, encoder_value, encoder_mask, W):
    global LAST_EXEC_NS, LAST_RESULTS
    from concourse.bass_utils import run_bass_kernel_spmd

    if "nc" not in _cache:
        _cache["nc"] = _build()
    nc = _cache["nc"]

    hidden = np.ascontiguousarray(hidden, dtype=np.float32)
    encoder_outputs = np.ascontiguousarray(encoder_outputs, dtype=np.float32)
    encoder_value = np.ascontiguousarray(encoder_value, dtype=np.float32)
    encoder_mask = np.ascontiguousarray(encoder_mask, dtype=np.float32)
    W = np.ascontiguousarray(W, dtype=np.float32)

    in_maps = []
    for c in range(NCORES):
        sl = slice(c * BPC, (c + 1) * BPC)
        in_maps.append({
            "hidT": np.ascontiguousarray(hidden[sl].transpose(0, 2, 1)),
            "w": W,
            "encT": np.ascontiguousarray(encoder_outputs[sl].transpose(0, 2, 1)),
            "val": encoder_value[sl],
            "mask": encoder_mask[sl][:, None, :],
        })

    trace = bool(int(os.environ.get("KERNEL_TRACE", "0")))
    res = run_bass_kernel_spmd(nc, in_maps, core_ids=list(range(NCORES)),
                               trace=trace)
    LAST_EXEC_NS = res.exec_time_ns
    LAST_RESULTS = res

    context = np.concatenate([res.results[c]["ctx"] for c in range(NCORES)], axis=0)
    attn_weights = np.concatenate([res.results[c]["aw"] for c in range(NCORES)],
                                  axis=0)
    attn_energies = np.concatenate([res.results[c]["ae"] for c in range(NCORES)],
                                   axis=0)
    return context, attn_weights, attn_energies


# revision 9
# speedup vs baseline: 1.6318x; 1.0095x over previous
"""Trainium2 Bass kernel for nn_Attention (general-score attention with
masked softmax), data-parallel over batch across 8 NeuronCores.

Math (per batch), matching the reference exactly for {0,1} float masks:
    raw[t,s]  = sum_e (hidden @ W)[t,e] * enc[s,e]       (associativity trick:
                (hidden @ W) @ enc^T  ==  hidden @ (enc @ W^T)^T, saves 25% FLOPs
                and avoids materializing proj)
    attn_energies = raw * mask            (mask in {0,1} so mask^2 == mask)
    e = exp(x - max_s x) * mask
    attn = e / (sum_s e + 1e-6)
    context = attn @ enc_value

Layouts: host marshals hidden^T (D,T) and enc^T (E,S) per batch so every
matmul contracts over the partition dim with zero on-device transposes,
except attn^T which is produced on-device via PE transpose.
All matmuls run in float32r (e8m11, 1 cycle/row at N>=512 vs 4 for f32);
measured end-to-end rel err ~1.2e-3.

Schedule: mm1 (per-dt-sliced loads for fast ramp) -> mm2 for ALL t-tiles
(freeing enc^T for the next batch mid-batch) with the softmax chains
pipelined on DVE/ACT behind the mm2 stream -> per t-tile PE transposes +
mm3. The PE stream never waits on a softmax chain in steady state.
"""
import os

import numpy as np

B, TRG, SRC, ENCD, TRGD = 16, 512, 1024, 1024, 1024
NCORES = 8
BPC = B // NCORES  # batches per core
P = 128
nD = TRGD // P   # 8 contraction tiles over d
nE = ENCD // P   # 8 over e
nS = SRC // P    # 8 over s
nT = TRG // P    # 4 t-tiles

_cache = {}

LAST_EXEC_NS = None
LAST_RESULTS = None


def _build():
    import concourse.mybir as mybir
    import concourse.tile as tile
    from concourse import bacc
    from concourse.masks import make_identity

    F32 = mybir.dt.float32
    F32R = mybir.dt.float32r
    ALU = mybir.AluOpType
    AXL = mybir.AxisListType
    ACT_EXP = mybir.ActivationFunctionType.Exp

    nc = bacc.Bacc("TRN2", target_bir_lowering=False, debug=False)

    hidT_d = nc.dram_tensor("hidT", (BPC, TRGD, TRG), F32R, kind="ExternalInput")
    w_d = nc.dram_tensor("w", (TRGD, ENCD), F32R, kind="ExternalInput")
    encT_d = nc.dram_tensor("encT", (BPC, ENCD, SRC), F32R, kind="ExternalInput")
    val_d = nc.dram_tensor("val", (BPC, SRC, TRGD), F32R, kind="ExternalInput")
    mask_d = nc.dram_tensor("mask", (BPC, 1, SRC), F32, kind="ExternalInput")
    ae_d = nc.dram_tensor("ae", (BPC, TRG, SRC), F32, kind="ExternalOutput")
    aw_d = nc.dram_tensor("aw", (BPC, TRG, SRC), F32R, kind="ExternalOutput")
    ctx_d = nc.dram_tensor("ctx", (BPC, TRG, TRGD), F32, kind="ExternalOutput")

    with tile.TileContext(nc) as tc:
        with (
            tc.tile_pool(name="const", bufs=1) as const,
            tc.tile_pool(name="wp", bufs=1) as wp,
            tc.tile_pool(name="big", bufs=1) as big,
            tc.tile_pool(name="sm", bufs=2) as sm,
            tc.tile_pool(name="xs", bufs=4) as xs,
            tc.tile_pool(name="psA", bufs=2, space="PSUM") as psA,
            tc.tile_pool(name="psB", bufs=3, space="PSUM") as psB,
        ):
            ident = const.tile([P, P], F32)
            make_identity(nc, ident[:])
            identr = const.tile([P, P], F32R)
            nc.vector.tensor_copy(identr[:], ident[:])

            # W resident for both batches, per-dt tiles for fast first-mm ramp
            w_sb = [wp.tile([P, ENCD], F32R, tag=f"w{i}", name=f"w_sb{i}")
                    for i in range(nD)]

            for b in range(BPC):
                hidT_sb = [big.tile([P, TRG], F32R, tag=f"hidT{i}",
                                     name=f"hidT_sb{i}") for i in range(nD)]
                # DMA issue order == consumption order: (w[i], hidT[i]) pairs
                # feed the dt-outer mm1 below as they land.
                for i in range(nD):
                    if b == 0:
                        nc.sync.dma_start(out=w_sb[i][:],
                                          in_=w_d[i * P:(i + 1) * P, :])
                    nc.sync.dma_start(out=hidT_sb[i][:],
                                      in_=hidT_d[b, i * P:(i + 1) * P, :])
                maskb = sm.tile([P, SRC], F32, tag="maskb")
                nc.sync.dma_start(out=maskb[:], in_=mask_d[b].to_broadcast((P, SRC)))
                encT_sb = big.tile([P, nE, SRC], F32R, tag="encT")
                for i in range(nE):
                    nc.sync.dma_start(out=encT_sb[:, i, :],
                                      in_=encT_d[b, i * P:(i + 1) * P, :])
                val_sb = big.tile([P, nS, TRGD], F32R, tag="val")
                for i in range(nS):
                    nc.sync.dma_start(out=val_sb[:, i, :],
                                      in_=val_d[b, i * P:(i + 1) * P, :])

                # mm1: H'^T[e,t] = sum_d W[d,e] * hidden^T[d,t], dt-OUTER over
                # 8 concurrent psum banks so each (w[dt], hidT[dt]) slice pair
                # is consumed as soon as its DMA lands.
                HpT = big.tile([P, nE, TRG], F32R, tag="HpT")
                mm1_ps = [psB.tile([P, SRC], F32, tag="ps_b",
                                   name=f"mm1ps{j}") for j in range(3)]
                mm1_ps2 = [psA.tile([P, TRG], F32, tag="ps_a",
                                    name=f"mm1ps2{j}") for j in range(2)]

                def et_psum(et):
                    if et < 6:
                        return mm1_ps[et // 2][:, (et % 2) * 512:(et % 2 + 1) * 512]
                    return mm1_ps2[et - 6][:]

                for dt in range(nD):
                    for et in range(nE):
                        nc.tensor.matmul(et_psum(et),
                                         w_sb[dt][:, et * P:(et + 1) * P],
                                         hidT_sb[dt][:],
                                         start=(dt == 0), stop=(dt == nD - 1))
                for et in range(nE):
                    nc.vector.tensor_copy(HpT[:, et, :], et_psum(et))

                # mm2 for all t-tiles; softmax chains pipeline behind the
                # PE stream on DVE/ACT.
                attns = []
                for tt in range(nT):
                    ts = slice(tt * P, (tt + 1) * P)
                    en_ps = psB.tile([P, SRC], F32, tag="ps_b")
                    for et in range(nE):
                        for h in range(2):
                            hs = slice(h * 512, (h + 1) * 512)
                            nc.tensor.matmul(en_ps[:, hs], HpT[:, et, ts],
                                             encT_sb[:, et, hs],
                                             start=(et == 0), stop=(et == nE - 1))

                    x = xs.tile([P, SRC], F32, tag="x")
                    nc.vector.tensor_mul(x[:], en_ps[:], maskb[:])
                    nc.sync.dma_start(out=ae_d[b, ts, :], in_=x[:])
                    negm = sm.tile([P, 1], F32, tag="negm")
                    nc.vector.tensor_reduce(negm[:], x[:], axis=AXL.X, op=ALU.max,
                                            negate=True)
                    ex = sm.tile([P, SRC], F32, tag="ex")
                    nc.scalar.activation(ex[:], x[:], ACT_EXP, bias=negm[:],
                                         scale=1.0)
                    rowsum = sm.tile([P, 1], F32, tag="rowsum")
                    nc.vector.scalar_tensor_tensor(ex[:], ex[:], 1.0, maskb[:],
                                                   op0=ALU.mult, op1=ALU.mult,
                                                   accum_out=rowsum[:])
                    emask = ex
                    z = sm.tile([P, 1], F32, tag="z")
                    nc.vector.tensor_scalar_add(z[:], rowsum[:], 1e-6)
                    rz = sm.tile([P, 1], F32, tag="rz")
                    nc.vector.reciprocal(rz[:], z[:])
                    attn = xs.tile([P, SRC], F32R, tag="attn")
                    nc.vector.tensor_scalar_mul(attn[:], emask[:], rz[:])
                    nc.sync.dma_start(out=aw_d[b, ts, :], in_=attn[:])
                    attns.append(attn)

                # per t-tile: attn^T via PE transpose, then mm3
                for tt in range(nT):
                    ts = slice(tt * P, (tt + 1) * P)
                    attn = attns[tt]
                    attnT = sm.tile([P, nS, P], F32R, tag="attnT")
                    for st in range(nS):
                        pt = psA.tile([P, TRG], F32R, tag="ps_a")
                        nc.tensor.transpose(pt[:, :P], attn[:, st * P:(st + 1) * P],
                                            identr[:])
                        if st % 2 == 0:
                            nc.vector.tensor_copy(attnT[:, st, :], pt[:, :P])
                        else:
                            nc.scalar.copy(attnT[:, st, :], pt[:, :P])

                    ctx_ps = psB.tile([P, TRGD], F32, tag="ps_b")
                    for st in range(nS):
                        for h in range(2):
                            hs = slice(h * 512, (h + 1) * 512)
                            nc.tensor.matmul(ctx_ps[:, hs], attnT[:, st, :],
                                             val_sb[:, st, hs],
                                             start=(st == 0), stop=(st == nS - 1))
                    ctx_sb = sm.tile([P, TRGD], F32, tag="ctx_sb")
                    nc.scalar.copy(ctx_sb[:], ctx_ps[:])
                    nc.sync.dma_start(out=ctx_d[b, ts, :], in_=ctx_sb[:])

    nc.compile()
    return nc


def kernel(hidden, encoder_outputs, encoder_value, encoder_mask, W):
    global LAST_EXEC_NS, LAST_RESULTS
    from concourse.bass_utils import run_bass_kernel_spmd

    if "nc" not in _cache:
        _cache["nc"] = _build()
    nc = _cache["nc"]

    hidden = np.ascontiguousarray(hidden, dtype=np.float32)
    encoder_outputs = np.ascontiguousarray(encoder_outputs, dtype=np.float32)
    encoder_value = np.ascontiguousarray(encoder_value, dtype=np.float32)
    encoder_mask = np.ascontiguousarray(encoder_mask, dtype=np.float32)
    W = np.ascontiguousarray(W, dtype=np.float32)

    in_maps = []
    for c in range(NCORES):
        sl = slice(c * BPC, (c + 1) * BPC)
        in_maps.append({
            "hidT": np.ascontiguousarray(hidden[sl].transpose(0, 2, 1)),
            "w": W,
            "encT": np.ascontiguousarray(encoder_outputs[sl].transpose(0, 2, 1)),
            "val": encoder_value[sl],
            "mask": encoder_mask[sl][:, None, :],
        })

    trace = bool(int(os.environ.get("KERNEL_TRACE", "0")))
    res = run_bass_kernel_spmd(nc, in_maps, core_ids=list(range(NCORES)),
                               trace=trace)
    LAST_EXEC_NS = res.exec_time_ns
    LAST_RESULTS = res

    context = np.concatenate([res.results[c]["ctx"] for c in range(NCORES)], axis=0)
    attn_weights = np.concatenate([res.results[c]["aw"] for c in range(NCORES)],
                                  axis=0)
    attn_energies = np.concatenate([res.results[c]["ae"] for c in range(NCORES)],
                                   axis=0)
    return context, attn_weights, attn_energies
